# revision 9
# baseline (speedup 1.0000x reference)
"""BitNet linear (y = (x @ sign(W).T + b) * mean(|W|)) on 8 trn2 NeuronCores.

Sharding: column-parallel — W is sharded along out_features across the 8
cores, x is replicated, each core produces out[:, shard] and the host
concatenates.

Device algorithm (per core): everything runs as fp8e4 (E4M3)
perf_mode=DoubleRow matmuls, which on trn2 stream at ~0.5 cycles per
output column (measured: a [128,2,512]x[128,2,512]->[128,512] DR matmul
takes ~256 PE cycles — 4x the bf16 MAC rate).  The contraction is laid
out as NSLOT = k8_chunks + 2*(K_CHUNKS-k8_chunks) fp8 "slots" of 128
k-partitions each, consumed 2 slots per DR matmul:

  - slots [0, k8_chunks): pair-packed — slot s holds fp8(x) for chunk s,
    the DR pair (2p, 2p+1) contracts 2 distinct k-chunks per pass.  All
    x-quantization error (~2.6e-2 per dim, scaled by sqrt(fraction))
    comes from here.
  - slots [k8_chunks, NSLOT) in (hi, lo) pairs per remaining chunk:
    hi = fp8(x), lo = fp8(x - hi), and the WEIGHT slice is duplicated
    across the two slots, so one DR matmul contracts hi+lo ~= x to
    ~8e-4 relative accuracy at half the cost of a bf16 pass.

  Weights are sign(W) in {-1,0,+1}: exact in fp8e4, so no weight error.
  Measured end-to-end L2 rel err at k8_chunks=16 (2048/4096 dims
  pair-packed): 1.88e-2 (gate: 2e-2).

  The host only does dtype casts + layout (x -> fp8 slot block-images,
  W^T -> bf16); sign(W), mean|W| (AllReduce across cores), the matmul,
  bias add and alpha scale all run on device.

  Per M-block (128 rows): 1 contiguous input DMA (fp8 slot image),
  NSLOT/2 * (N_shard/512) DR matmuls into 4 PSUM banks, fused
  scale(alpha)+bias PSUM->SBUF op, 1 output DMA.  Next block's input
  DMA is emitted before this block's PSUM drain so the PE never
  starves at block boundaries.
"""

import numpy as np
import ml_dtypes

import concourse.bass as bass
import concourse.mybir as mybir
import concourse.tile as tile
from concourse.bass import ds
from concourse.vector_clock import ScopedClock

# ---------------------------------------------------------------------------
# Compatibility patch: the pinned walrus (neuronxcc) in this container only
# supports ONE ge-wait per instruction and no eq-waits; the concourse Tile
# tail emits a Drain with multiple waits plus an eq-wait barrier butterfly
# ("Too many sync wait commands").  Replace the tail with one-wait-per-nop
# splitting and the NRT-expanded PSEUDO_SYNC_BARRIER (the pre-butterfly
# mechanism this walrus/NRT pair supports).
# ---------------------------------------------------------------------------


def _compat_drain_and_barrier(self, tick_clock, wait_clock):
    nc = self.nc
    coll = nc.sync.nop(nofuse=True)
    wait_clock.add_sem_waits(coll.ins, ScopedClock({None: tick_clock.global_clock}))
    si = coll.ins.sync_info
    if si is not None:
        waits = list(si.on_wait)
        if len(waits) > 1:
            coll.ins.sync_info = mybir.SyncInfo(
                on_wait=[waits[0]], on_update=list(si.on_update)
            )
            for w in waits[1:]:
                extra = nc.sync.nop(nofuse=True)
                extra.ins.sync_info = mybir.SyncInfo(on_wait=[w], on_update=[])
    for eng in nc.engines.values():
        eng.drain()
    nc._nrt_pseudo_barrier()
    popped = nc._tile_sem_poison_stack.pop()
    assert popped is self._sem_poison
    nc.clear_and_free_semaphores(list(self.sems.allocated().values()))
    nc._nrt_pseudo_barrier()


tile.TileContext._drain_and_barrier = _compat_drain_and_barrier

_legalize_ctr = [0]


def legalize_waits(nc):
    """Split instructions carrying more than the HW-supported number of sem
    waits (1; EventSemaphore: 2) into preceding one-wait NoOps on the same
    engine — semantically identical, encodable by the pinned walrus."""
    import bass_rust

    for f in nc.m.functions:
        for bb in f.blocks:
            il = bb.instructions
            i = 0
            while i < len(il):
                ins = il[i]
                si = ins.sync_info
                waits = list(si.on_wait) if si is not None else []
                limit = 2 if type(ins).__name__ == "InstEventSemaphore" else 1
                if len(waits) > limit:
                    keep = waits[-limit:]
                    spill = waits[:-limit]
                    for w in spill:
                        _legalize_ctr[0] += 1
                        nop = bass_rust.InstNoOp(
                            name=f"I-lw{_legalize_ctr[0]}", ins=[], outs=[]
                        )
                        nop.engine = ins.engine
                        nop.sync_info = mybir.SyncInfo(on_wait=[w], on_update=[])
                        il.insert(i, nop)
                        i += 1
                    ins.sync_info = mybir.SyncInfo(
                        on_wait=keep, on_update=list(si.on_update)
                    )
                i += 1


def elide_redundant_ldweights(nc):
    """Drop InstLdweights that reload the exact weights already sitting in
    the PE array.  bass lowers every InstMatmult to an Ldweights+Matmult
    pair; consecutive matmuls sharing one stationary tile reload it each
    time (~107ns of PE time apiece).  Two Ldweights with no other Ldweights
    between them and the same (tile name, offset, pattern) provably load
    identical content — tile names are unique per pool.tile() call and each
    tile is written before its first consumer only.  Elided instructions
    carrying semaphore waits/updates become NoOps to preserve sync."""
    import bass_rust

    n_elided = 0
    for f in nc.m.functions:
        for bb in f.blocks:
            il = bb.instructions
            last_key = None
            for i in range(len(il)):
                ins = il[i]
                nm = type(ins).__name__
                if nm != "InstLdweights":
                    continue
                a = ins.ins[0]
                bap = getattr(a, "bass_ap", None)
                if bap is None:
                    last_key = None
                    continue
                key = (
                    bap.tensor.name,
                    bap.offset,
                    str(bap.ap),
                    ins.perf_mode,
                    ins.is_transpose,
                    ins.tile_position,
                )
                if key == last_key:
                    si = ins.sync_info
                    has_sync = si is not None and (
                        list(si.on_wait) or list(si.on_update)
                    )
                    nop = bass_rust.InstNoOp(name=f"{ins.name}-eld", ins=[], outs=[])
                    nop.engine = ins.engine
                    if has_sync:
                        nop.sync_info = mybir.SyncInfo(
                            on_wait=list(si.on_wait), on_update=list(si.on_update)
                        )
                    il[i] = nop
                    n_elided += 1
                else:
                    last_key = key
    return n_elided


F32 = mybir.dt.float32
BF16 = mybir.dt.bfloat16
F8 = mybir.dt.float8e4

P = 128  # partitions
K8_CHUNKS_DEFAULT = 16  # fp8 region size in 128-chunks (of K/128 total)


def build_bitnet_nc(
    M: int,
    K: int,
    N_shard: int,
    n_total_weight: int,
    n_cores: int = 8,
    k8_chunks: int = K8_CHUNKS_DEFAULT,
    legalize: bool = True,
    reps: int = 1,
    skip_cc: bool = False,
    fuse_bias: bool = True,
):
    """Build the per-core Bass program.

    M: rows of x (B*S), K: in_features, N_shard: out_features per core.
    n_total_weight: total element count of the full W (for mean(|W|)).
    k8_chunks: leading 128-chunks of K computed in fp8-DoubleRow (even).
    """
    assert M % P == 0 and K % P == 0
    K_CHUNKS = K // P
    K8C = k8_chunks
    assert 0 <= K8C <= K_CHUNKS and K8C % 2 == 0
    KBC = K_CHUNKS - K8C
    NSLOT = K8C + 2 * KBC  # fp8 slots; consumed 2 per DoubleRow matmul
    N_TILE = min(512, N_shard)
    assert N_shard % N_TILE == 0
    NB = N_shard // N_TILE
    M_BLOCKS = M // P

    nc = bass.Bass(num_devices=n_cores)
    # host-prepared per-block SBUF images: [block, partition(k%128), slot, m]
    xs_d = nc.declare_dram_parameter("xs", [M_BLOCKS, P, NSLOT, P], F8,
                                     isOutput=False)
    wT_d = nc.declare_dram_parameter("wT", [K, N_shard], BF16, isOutput=False)
    bias_d = nc.declare_dram_parameter("bias", [N_shard], F32, isOutput=False)
    out_d = nc.declare_dram_parameter("out", [M, N_shard], F32, isOutput=True)

    DR = mybir.MatmulPerfMode.DoubleRow

    with tile.TileContext(nc) as tc:
        wq_pool = tc.tile_pool(name="wq", bufs=1)
        wstage = tc.tile_pool(name="wstage", bufs=2)
        small = tc.tile_pool(name="small", bufs=1)
        xs_pool = tc.tile_pool(name="xsp", bufs=2)
        out_pool = tc.tile_pool(name="outp", bufs=2)
        # 7 rotating PSUM slots for the 4 accumulation chains per block (all
        # four must be live at once or the scheduler serializes the chains
        # into waves, repeating every LDWEIGHTS for each wave); +1 bank for
        # the alpha partition-reduce = all 8 banks.
        psum_pool = tc.tile_pool(name="psum", bufs=7, space="PSUM")
        apsum_pool = tc.tile_pool(name="apsum", bufs=1, space="PSUM")
        dram = tc.tile_pool(name="dram", bufs=1, space="DRAM")

        with (
            wq_pool as wq_p,
            wstage as wst_p,
            small as small_p,
            xs_pool as xs_p,
            out_pool as out_p,
            psum_pool as ps_p,
            apsum_pool as aps_p,
            dram as dram_p,
        ):
            # ---------------- Phase A: sign(W) + |W| partial sums ----------
            # w^T arrives bf16; sign is exact in fp8 ({-1,0,+1}).  Chunk k
            # fills slot k (pair region) or slots K8C+2(k-K8C)(+1) twice
            # (hi/lo region: the weight is the same for both x slots).
            wq = wq_p.tile([P, NSLOT, N_shard], F8)
            acc = small_p.tile([P, K_CHUNKS], F32)
            abs_dump = small_p.tile([P, N_shard], F32)
            for k in range(K_CHUNKS):
                wst = wst_p.tile([P, N_shard], BF16, tag="wst")
                nc.sync.dma_start(wst[:], wT_d[k * P : (k + 1) * P, :])
                # per-chunk |W| sum on ScalarE while DVE does the sign clamp
                nc.scalar.activation(
                    abs_dump[:],
                    wst[:],
                    mybir.ActivationFunctionType.Abs,
                    accum_out=acc[:, k : k + 1],
                )
                # sign via clamp: s = max(min(w * 1e30, 1), -1), exact
                # {-1, 0, +1}; run in bf16 (no overflow: bf16 exponent range
                # matches fp32), then cast to the fp8 slot(s).
                sgn = wst_p.tile([P, N_shard], BF16, tag="sgn")
                nc.vector.tensor_scalar(
                    sgn[:], wst[:], 1e30, 1.0,
                    mybir.AluOpType.mult, mybir.AluOpType.min,
                )
                nc.vector.tensor_scalar(
                    sgn[:], sgn[:], -1.0, None, mybir.AluOpType.max
                )
                if k < K8C:
                    nc.vector.tensor_copy(wq[:, k, :], sgn[:])
                else:
                    s0 = K8C + 2 * (k - K8C)
                    nc.vector.tensor_copy(wq[:, s0, :], sgn[:])
                    nc.vector.tensor_copy(wq[:, s0 + 1, :], sgn[:])

            # ---------------- Phase B: alpha = mean|W| over all cores ------
            asum = small_p.tile([P, 1], F32)
            nc.vector.reduce_sum(asum[:], acc[:], axis=mybir.AxisListType.X)
            ones_pp = small_p.tile([P, P], F32)
            nc.vector.memset(ones_pp[:], 1.0)
            aps = aps_p.tile([P, 1], F32)
            # ones^T @ asum: sum over partitions, broadcast to all partitions
            nc.tensor.matmul(aps[:], ones_pp[:], asum[:], start=True, stop=True)
            part_sum = small_p.tile([P, 1], F32)
            nc.vector.tensor_copy(part_sum[:], aps[:])

            cc_in = dram_p.tile([P, 1], F32)
            cc_out = dram_p.tile(
                [P, 1], F32, addr_space="Shared" if n_cores > 4 else "Local"
            )
            nc.sync.dma_start(cc_in[:], part_sum[:])
            if skip_cc:
                nc.sync.dma_start(cc_out[:], cc_in[:])
            else:
                nc.gpsimd.collective_compute(
                    "AllReduce",
                    mybir.AluOpType.add,
                    replica_groups=[list(range(n_cores))],
                    ins=[cc_in.opt()],
                    outs=[cc_out.opt()],
                )
            gsum = small_p.tile([P, 1], F32)
            nc.sync.dma_start(gsum[:], cc_out[:])
            alpha = small_p.tile([P, 1], F32)
            nc.vector.tensor_scalar_mul(alpha[:], gsum[:], 1.0 / float(n_total_weight))

            # bias: build a [128, N_shard] bf16 broadcast of bias*alpha ONCE
            # (ones-matmul broadcast); fused into the per-block PSUM->SBUF op
            bias_sb = small_p.tile([1, N_shard], F32)
            nc.sync.dma_start(bias_sb[:], bias_d[None, :])
            ones_row = small_p.tile([1, P], F32)
            nc.vector.memset(ones_row[:], 1.0)
            bias_bc = small_p.tile([P, N_shard], BF16)
            if fuse_bias:
                nc.vector.tensor_scalar_mul(bias_sb[:], bias_sb[:], alpha[:1, :])
                for n in range(NB):
                    bps = ps_p.tile([P, N_TILE], F32, tag="ps", name=f"bps{n}")
                    nc.tensor.matmul(
                        bps[:],
                        ones_row[:],
                        bias_sb[:, ds(n * N_TILE, N_TILE)],
                        start=True,
                        stop=True,
                    )
                    nc.vector.tensor_copy(bias_bc[:, ds(n * N_TILE, N_TILE)], bps[:])

            # ---------------- Phase C: main matmul -------------------------
            total_blocks = reps * M_BLOCKS
            NSTAT = NSLOT // 2  # DR stationaries per block

            def emit_in_dma(m, tag):
                xst = xs_p.tile([P, NSLOT, P], F8, tag="xs", name=f"xs{tag}")
                nc.sync.dma_start(xst[:], xs_d[m])
                return xst

            pending = emit_in_dma(0, "b0")
            for bi in range(total_blocks):
                m = bi % M_BLOCKS
                xst = pending

                psums = [
                    ps_p.tile([P, N_TILE], F32, tag="ps", name=f"ps{n}")
                    for n in range(NB)
                ]
                for s in range(NSTAT):
                    lhsT = xst[:, 2 * s : 2 * s + 2, :]
                    last = s == NSTAT - 1
                    for n in range(NB):
                        nc.tensor.matmul(
                            psums[n][:],
                            lhsT,
                            wq[:, 2 * s : 2 * s + 2, ds(n * N_TILE, N_TILE)],
                            start=s == 0,
                            stop=last,
                            perf_mode=DR,
                        )

                # next block's input DMA BEFORE this block's PSUM drain so
                # the DMA queue isn't stuck behind the output store
                if bi + 1 < total_blocks:
                    pending = emit_in_dma((bi + 1) % M_BLOCKS, f"b{bi + 1}")

                osb = out_p.tile([P, N_shard], F32, tag="osb")
                for n in range(NB):
                    if fuse_bias:
                        nc.vector.scalar_tensor_tensor(
                            osb[:, ds(n * N_TILE, N_TILE)],
                            psums[n][:],
                            alpha[:],
                            bias_bc[:, ds(n * N_TILE, N_TILE)],
                            mybir.AluOpType.mult,
                            mybir.AluOpType.add,
                        )
                    else:
                        nc.vector.tensor_scalar_mul(
                            osb[:, ds(n * N_TILE, N_TILE)], psums[n][:], alpha[:]
                        )
                nc.sync.dma_start(out_d[m * P : (m + 1) * P, :], osb[:])

    if legalize:
        legalize_waits(nc)  # required for walrus; CoreSim chokes on raw NoOps
    elide_redundant_ldweights(nc)
    return nc


def _host_prepare(x: np.ndarray, weight: np.ndarray, bias: np.ndarray,
                  n_cores: int, k8_chunks: int):
    """Host-side dtype casts + layout (no arithmetic beyond rounding):
    x -> per-block fp8/bf16 SBUF images (shared across cores), W^T -> bf16
    per-core shards."""
    lead_shape = x.shape[:-1]
    K = x.shape[-1]
    N = weight.shape[0]
    M = int(np.prod(lead_shape))
    assert weight.shape == (N, K) and bias.shape == (N,)
    assert N % n_cores == 0
    N_shard = N // n_cores
    K8 = k8_chunks * P
    KBC = K // P - k8_chunks
    NSLOT = k8_chunks + 2 * KBC
    M_BLOCKS = M // P

    x2 = np.ascontiguousarray(x.reshape(M, K).astype(np.float32, copy=False))
    f8 = ml_dtypes.float8_e4m3
    xs = np.empty((M_BLOCKS, P, NSLOT, P), dtype=f8)
    # [mb, m, c, p] -> [mb, p, c, m]
    if k8_chunks:
        a = x2[:, :K8].astype(f8)
        xs[:, :, :k8_chunks, :] = a.reshape(M_BLOCKS, P, k8_chunks, P).transpose(
            0, 3, 2, 1
        )
    if KBC:
        hi = x2[:, K8:].astype(f8)
        lo = (x2[:, K8:] - hi.astype(np.float32)).astype(f8)
        xs[:, :, k8_chunks::2, :] = hi.reshape(M_BLOCKS, P, KBC, P).transpose(
            0, 3, 2, 1
        )
        xs[:, :, k8_chunks + 1 :: 2, :] = lo.reshape(
            M_BLOCKS, P, KBC, P
        ).transpose(0, 3, 2, 1)

    w = weight.astype(np.float32, copy=False)
    in_maps = []
    for c in range(n_cores):
        wTc = np.ascontiguousarray(
            w[c * N_shard : (c + 1) * N_shard, :].T.astype(ml_dtypes.bfloat16)
        )
        bc = np.ascontiguousarray(bias[c * N_shard : (c + 1) * N_shard]).astype(
            np.float32, copy=False
        )
        in_maps.append({"xs": xs, "wT": wTc, "bias": bc})
    return in_maps, M, K, N, N_shard, lead_shape


def run_bitnet(
    x: np.ndarray,
    weight: np.ndarray,
    bias: np.ndarray,
    n_cores: int = 8,
    k8_chunks: int = K8_CHUNKS_DEFAULT,
    trace: bool = False,
):
    """Host driver: shard, run on n_cores, gather. x: [..., K], weight: [N, K]."""
    from concourse.bass_utils import run_bass_kernel_spmd

    in_maps, M, K, N, N_shard, lead_shape = _host_prepare(
        x, weight, bias, n_cores, k8_chunks
    )
    nc = build_bitnet_nc(M, K, N_shard, N * K, n_cores=n_cores,
                         k8_chunks=k8_chunks)
    res = run_bass_kernel_spmd(
        nc, in_maps, core_ids=list(range(n_cores)), trace=trace
    )
    out = np.empty((M, N), dtype=np.float32)
    for c in range(n_cores):
        out[:, c * N_shard : (c + 1) * N_shard] = res.results[c]["out"]
    return out.reshape(*lead_shape, N), res


_RUNNER_CACHE: dict = {}


def _make_runner(nc, n_cores, in_map_names=None):
    """Compile a sharded PJRT executor for the given Bass program."""
    import jax
    import jax.numpy as jnp
    from jax.sharding import Mesh, NamedSharding, PartitionSpec
    from jax.experimental.shard_map import shard_map

    from concourse import bass2jax
    from concourse.bass2jax import _bass_exec_p, partition_id_tensor

    bass2jax.install_neuronx_cc_hook()
    partition_name = nc.partition_id_tensor.name if nc.partition_id_tensor else None
    in_names, out_names, out_avals, zero_outs = [], [], [], []
    for alloc in nc.m.functions[0].allocations:
        if not isinstance(alloc, mybir.MemoryLocationSet):
            continue
        name = alloc.memorylocations[0].name
        if alloc.kind == "ExternalInput":
            if name != partition_name:
                in_names.append(name)
        elif alloc.kind == "ExternalOutput":
            shape = tuple(alloc.tensor_shape)
            dtype = mybir.dt.np(alloc.dtype)
            out_names.append(name)
            out_avals.append(jax.core.ShapedArray(shape, dtype))
            zero_outs.append(np.zeros(shape, dtype))
    n_params = len(in_names)
    n_outs = len(out_avals)
    param_names = list(in_names)
    in_names = in_names + out_names
    if partition_name is not None:
        in_names.append(partition_name)
    donate = tuple(range(n_params, n_params + n_outs))

    def _body(*args):
        operands = list(args)
        if partition_name is not None:
            operands.append(partition_id_tensor())
        return tuple(
            _bass_exec_p.bind(
                *operands,
                out_avals=tuple(out_avals),
                in_names=tuple(in_names),
                out_names=tuple(out_names),
                lowering_input_output_aliases=(),
                sim_require_finite=True,
                sim_require_nnan=True,
                nc=nc,
            )
        )

    devices = jax.devices()[:n_cores]
    mesh = Mesh(np.asarray(devices), ("core",))
    sh = NamedSharding(mesh, PartitionSpec("core"))
    sharded = jax.jit(
        shard_map(
            _body,
            mesh=mesh,
            in_specs=(PartitionSpec("core"),) * (n_params + n_outs),
            out_specs=(PartitionSpec("core"),) * len(out_names),
            check_rep=False,
        ),
        donate_argnums=donate,
        keep_unused=True,
    )
    zfns = [
        jax.jit(
            lambda shp=(n_cores * z.shape[0], *z.shape[1:]),
            dt=z.dtype: jnp.zeros(shp, dt),
            out_shardings=sh,
        )
        for z in zero_outs
    ]
    return sharded, param_names, out_names, out_avals, sh, zfns


def _cached_pjrt_run(M, K, N_shard, n_cores, k8_chunks, in_maps):
    """Compile-once-per-shape PJRT executor; repeat kernel() calls skip the
    multi-minute NEFF rebuild and only pay transfer + execution."""
    import jax

    key = (M, K, N_shard, n_cores, k8_chunks)
    if key not in _RUNNER_CACHE:
        nc = build_bitnet_nc(M, K, N_shard, N_shard * n_cores * K,
                             n_cores=n_cores, k8_chunks=k8_chunks)
        _RUNNER_CACHE[key] = _make_runner(nc, n_cores)

    sharded, param_names, out_names, out_avals, sh, zfns = _RUNNER_CACHE[key]

    concat_in = [
        jax.device_put(
            np.concatenate(
                [np.asarray(in_maps[c][nm]) for c in range(n_cores)], 0
            ),
            sh,
        )
        for nm in param_names
    ]
    out_arrs = sharded(*concat_in, *[f() for f in zfns])
    oi = out_names.index("out")
    glob = np.asarray(out_arrs[oi]).reshape(n_cores, *out_avals[oi].shape)
    return [glob[c] for c in range(n_cores)]


def kernel(x: np.ndarray, weight: np.ndarray, bias: np.ndarray) -> np.ndarray:
    n_cores = 8
    k8_chunks = K8_CHUNKS_DEFAULT
    in_maps, M, K, N, N_shard, lead_shape = _host_prepare(
        x, weight, bias, n_cores, k8_chunks
    )
    shards = _cached_pjrt_run(M, K, N_shard, n_cores, k8_chunks, in_maps)
    out = np.empty((M, N), dtype=np.float32)
    for c in range(n_cores):
        out[:, c * N_shard : (c + 1) * N_shard] = shards[c]
    return out.reshape(*lead_shape, N)


def run_bitnet_timed(
    x: np.ndarray,
    weight: np.ndarray,
    bias: np.ndarray,
    n_cores: int = 8,
    nsplits: int = 2,  # kept for test.py signature compat; unused
    reps: int = 4,
    rounds: int = 6,
    k8_chunks: int = K8_CHUNKS_DEFAULT,
):
    """Like run_bitnet, but measures HW time via the reps-difference method:
    build the kernel once plain and once with the main loop unrolled `reps`
    times, time single dispatches of each (min over `rounds`), and divide the
    delta by reps-1.  This cancels the multi-ms, noisy axon dispatch floor.
    Returns (out, per_exec_seconds, diag)."""
    import time

    import jax

    in_maps, M, K, N, N_shard, lead_shape = _host_prepare(
        x, weight, bias, n_cores, k8_chunks
    )

    def runner_for(reps_):
        nc = build_bitnet_nc(M, K, N_shard, N * K, n_cores=n_cores,
                             k8_chunks=k8_chunks, reps=reps_)
        sharded, param_names, out_names, out_avals, sh, zfns = _make_runner(
            nc, n_cores
        )
        concat_in = [
            jax.device_put(
                np.concatenate(
                    [np.asarray(in_maps[c][nm]) for c in range(n_cores)], 0
                ),
                sh,
            )
            for nm in param_names
        ]

        def run_once():
            z = [f() for f in zfns]
            jax.block_until_ready(z)
            t0 = time.perf_counter()
            o = sharded(*concat_in, *z)
            jax.block_until_ready(o)
            return time.perf_counter() - t0, o

        return run_once, out_names, out_avals

    run1, out_names, out_avals = runner_for(1)
    t_warm, out_arrs = run1()  # includes NEFF compile+load

    runR, _, _ = runner_for(reps)
    runR()  # warmup/compile

    t1s, tRs = [], []
    for _ in range(rounds):
        t1s.append(run1()[0])
        tRs.append(runR()[0])
    t1 = min(t1s)
    tR = min(tRs)
    per_exec = (tR - t1) / (reps - 1)
    diag = {"t1_min": t1, "tR_min": tR, "t1s": t1s, "tRs": tRs}

    oi = out_names.index("out")
    glob = np.asarray(out_arrs[oi]).reshape(n_cores, M, N_shard)
    out = np.empty((M, N), dtype=np.float32)
    for c in range(n_cores):
        out[:, c * N_shard : (c + 1) * N_shard] = glob[c]
    return out.reshape(*lead_shape, N), per_exec, diag


# revision 16
# speedup vs baseline: 1.2601x; 1.2601x over previous
"""BitNet linear (y = (x @ sign(W).T + b) * mean(|W|)) on 8 trn2 NeuronCores.

Sharding: column-parallel — W is sharded along out_features across the 8
cores, x is replicated, each core produces out[:, shard] and the host
concatenates.

Device algorithm (per core): everything runs as fp8e4 (E4M3)
perf_mode=DoubleRow matmuls, which on trn2 stream at ~0.5 cycles per
output column (measured: a [128,2,512]x[128,2,512]->[128,512] DR matmul
takes ~256 PE cycles — 4x the bf16 MAC rate).  The contraction is laid
out as NSLOT = k8_chunks + 2*(K_CHUNKS-k8_chunks) fp8 "slots" of 128
k-partitions each, consumed 2 slots per DR matmul:

  - slots [0, k8_chunks): pair-packed — slot s holds fp8(x) for chunk s,
    the DR pair (2p, 2p+1) contracts 2 distinct k-chunks per pass.  All
    x-quantization error (~2.6e-2 per dim, scaled by sqrt(fraction))
    comes from here.
  - slots [k8_chunks, NSLOT) in (hi, lo) pairs per remaining chunk:
    hi = fp8(x), lo = fp8(x - hi), and the WEIGHT slice is duplicated
    across the two slots, so one DR matmul contracts hi+lo ~= x to
    ~8e-4 relative accuracy at half the cost of a bf16 pass.

  Weights are sign(W) in {-1,0,+1}: exact in fp8e4, so no weight error.
  Measured end-to-end L2 rel err at k8_chunks=16 (2048/4096 dims
  pair-packed): 1.88e-2 (gate: 2e-2).

  The host only does dtype casts + layout (x -> fp8 slot block-images,
  W^T -> bf16); sign(W), mean|W| (AllReduce across cores), the matmul,
  bias add and alpha scale all run on device.

  Per M-block (128 rows): 1 contiguous input DMA (fp8 slot image),
  NSLOT/2 * (N_shard/512) DR matmuls into 4 PSUM banks, fused
  scale(alpha)+bias PSUM->SBUF op, 1 output DMA.  Next block's input
  DMA is emitted before this block's PSUM drain so the PE never
  starves at block boundaries.
"""

import numpy as np
import ml_dtypes

import concourse.bass as bass
import concourse.mybir as mybir
import concourse.tile as tile
from concourse.bass import ds
from concourse.vector_clock import ScopedClock

# ---------------------------------------------------------------------------
# Compatibility patch: the pinned walrus (neuronxcc) in this container only
# supports ONE ge-wait per instruction and no eq-waits; the concourse Tile
# tail emits a Drain with multiple waits plus an eq-wait barrier butterfly
# ("Too many sync wait commands").  Replace the tail with one-wait-per-nop
# splitting and the NRT-expanded PSEUDO_SYNC_BARRIER (the pre-butterfly
# mechanism this walrus/NRT pair supports).
# ---------------------------------------------------------------------------


def _compat_drain_and_barrier(self, tick_clock, wait_clock):
    nc = self.nc
    coll = nc.sync.nop(nofuse=True)
    wait_clock.add_sem_waits(coll.ins, ScopedClock({None: tick_clock.global_clock}))
    si = coll.ins.sync_info
    if si is not None:
        waits = list(si.on_wait)
        if len(waits) > 1:
            coll.ins.sync_info = mybir.SyncInfo(
                on_wait=[waits[0]], on_update=list(si.on_update)
            )
            for w in waits[1:]:
                extra = nc.sync.nop(nofuse=True)
                extra.ins.sync_info = mybir.SyncInfo(on_wait=[w], on_update=[])
    for eng in nc.engines.values():
        eng.drain()
    nc._nrt_pseudo_barrier()
    popped = nc._tile_sem_poison_stack.pop()
    assert popped is self._sem_poison
    nc.clear_and_free_semaphores(list(self.sems.allocated().values()))
    nc._nrt_pseudo_barrier()


tile.TileContext._drain_and_barrier = _compat_drain_and_barrier

_legalize_ctr = [0]


def legalize_waits(nc):
    """Split instructions carrying more than the HW-supported number of sem
    waits (1; EventSemaphore: 2) into preceding one-wait NoOps on the same
    engine — semantically identical, encodable by the pinned walrus."""
    import bass_rust

    for f in nc.m.functions:
        for bb in f.blocks:
            il = bb.instructions
            i = 0
            while i < len(il):
                ins = il[i]
                si = ins.sync_info
                waits = list(si.on_wait) if si is not None else []
                limit = 2 if type(ins).__name__ == "InstEventSemaphore" else 1
                if len(waits) > limit:
                    keep = waits[-limit:]
                    spill = waits[:-limit]
                    for w in spill:
                        _legalize_ctr[0] += 1
                        nop = bass_rust.InstNoOp(
                            name=f"I-lw{_legalize_ctr[0]}", ins=[], outs=[]
                        )
                        nop.engine = ins.engine
                        nop.sync_info = mybir.SyncInfo(on_wait=[w], on_update=[])
                        il.insert(i, nop)
                        i += 1
                    ins.sync_info = mybir.SyncInfo(
                        on_wait=keep, on_update=list(si.on_update)
                    )
                i += 1


def elide_redundant_ldweights(nc):
    """Drop InstLdweights that reload the exact weights already sitting in
    the PE array.  bass lowers every InstMatmult to an Ldweights+Matmult
    pair; consecutive matmuls sharing one stationary tile reload it each
    time (~107ns of PE time apiece).  Two Ldweights with no other Ldweights
    between them and the same (tile name, offset, pattern) provably load
    identical content — tile names are unique per pool.tile() call and each
    tile is written before its first consumer only.  Elided instructions
    carrying semaphore waits/updates become NoOps to preserve sync."""
    import bass_rust

    n_elided = 0
    for f in nc.m.functions:
        for bb in f.blocks:
            il = bb.instructions
            last_key = None
            keep = []
            for ins in il:
                nm = type(ins).__name__
                if nm != "InstLdweights":
                    keep.append(ins)
                    continue
                a = ins.ins[0]
                bap = getattr(a, "bass_ap", None)
                if bap is None:
                    last_key = None
                    keep.append(ins)
                    continue
                key = (
                    bap.tensor.name,
                    bap.offset,
                    str(bap.ap),
                    ins.perf_mode,
                    ins.is_transpose,
                    ins.tile_position,
                )
                if key == last_key:
                    si = ins.sync_info
                    has_sync = si is not None and (
                        list(si.on_wait) or list(si.on_update)
                    )
                    n_elided += 1
                    if has_sync:
                        # keep the semaphore behavior as a NoOp
                        nop = bass_rust.InstNoOp(
                            name=f"{ins.name}-eld", ins=[], outs=[]
                        )
                        nop.engine = ins.engine
                        nop.sync_info = mybir.SyncInfo(
                            on_wait=list(si.on_wait), on_update=list(si.on_update)
                        )
                        keep.append(nop)
                    # else: drop the instruction entirely — a sync-free NoOp
                    # still costs a PE sequencer issue slot
                else:
                    last_key = key
                    keep.append(ins)
            il[:] = keep
    return n_elided


F32 = mybir.dt.float32
BF16 = mybir.dt.bfloat16
F8 = mybir.dt.float8e4

P = 128  # partitions
K8_CHUNKS_DEFAULT = 16  # fp8 region size in 128-chunks (of K/128 total)


def build_bitnet_nc(
    M: int,
    K: int,
    N_shard: int,
    n_total_weight: int,
    n_cores: int = 8,
    k8_chunks: int = K8_CHUNKS_DEFAULT,
    legalize: bool = True,
    reps: int = 1,
    skip_cc: bool = False,
    fuse_bias: bool = True,
    no_drain: bool = False,   # timing-only: skip PSUM->SBUF + out DMA
    no_indma: bool = False,   # timing-only: single memset xs instead of DMA
):
    """Build the per-core Bass program.

    M: rows of x (B*S), K: in_features, N_shard: out_features per core.
    n_total_weight: total element count of the full W (for mean(|W|)).
    k8_chunks: leading 128-chunks of K computed in fp8-DoubleRow (even).
    """
    assert M % P == 0 and K % P == 0
    K_CHUNKS = K // P
    K8C = k8_chunks
    assert 0 <= K8C <= K_CHUNKS and K8C % 2 == 0
    KBC = K_CHUNKS - K8C
    NSLOT = K8C + 2 * KBC  # fp8 slots; consumed 2 per DoubleRow matmul
    N_TILE = min(512, N_shard)
    assert N_shard % N_TILE == 0
    NB = N_shard // N_TILE
    M_BLOCKS = M // P

    nc = bass.Bass(num_devices=n_cores)
    # host-prepared per-block SBUF images: [block, partition(k%128), slot, m]
    xs_d = nc.declare_dram_parameter("xs", [M_BLOCKS, P, NSLOT, P], F8,
                                     isOutput=False)
    wT_d = nc.declare_dram_parameter("wT", [K, N_shard], BF16, isOutput=False)
    bias_d = nc.declare_dram_parameter("bias", [N_shard], F32, isOutput=False)
    out_d = nc.declare_dram_parameter("out", [M, N_shard], F32, isOutput=True)

    DR = mybir.MatmulPerfMode.DoubleRow

    with tile.TileContext(nc) as tc:
        wq_pool = tc.tile_pool(name="wq", bufs=1)
        wstage = tc.tile_pool(name="wstage", bufs=2)
        small = tc.tile_pool(name="small", bufs=1)
        xs_pool = tc.tile_pool(name="xsp", bufs=3)
        out_pool = tc.tile_pool(name="outp", bufs=2)
        # 7 rotating PSUM slots for the 4 accumulation chains per block (all
        # four must be live at once or the scheduler serializes the chains
        # into waves, repeating every LDWEIGHTS for each wave); +1 bank for
        # the alpha partition-reduce = all 8 banks.
        psum_pool = tc.tile_pool(name="psum", bufs=7, space="PSUM")
        apsum_pool = tc.tile_pool(name="apsum", bufs=1, space="PSUM")
        dram = tc.tile_pool(name="dram", bufs=1, space="DRAM")

        with (
            wq_pool as wq_p,
            wstage as wst_p,
            small as small_p,
            xs_pool as xs_p,
            out_pool as out_p,
            psum_pool as ps_p,
            apsum_pool as aps_p,
            dram as dram_p,
        ):
            # ---------------- Phase A: sign(W) + |W| partial sums ----------
            # w^T arrives bf16; sign is exact in fp8 ({-1,0,+1}).  Chunk k
            # fills slot k (pair region) or slots K8C+2(k-K8C)(+1) twice
            # (hi/lo region: the weight is the same for both x slots).
            wq = wq_p.tile([P, NSLOT, N_shard], F8)
            acc = small_p.tile([P, K_CHUNKS], F32)
            abs_dump = small_p.tile([P, N_shard], F32)
            for k in range(K_CHUNKS):
                wst = wst_p.tile([P, N_shard], BF16, tag="wst")
                nc.sync.dma_start(wst[:], wT_d[k * P : (k + 1) * P, :])
                # per-chunk |W| sum on ScalarE while DVE does the sign clamp
                nc.scalar.activation(
                    abs_dump[:],
                    wst[:],
                    mybir.ActivationFunctionType.Abs,
                    accum_out=acc[:, k : k + 1],
                )
                # sign via clamp: s = max(min(w * 1e30, 1), -1), exact
                # {-1, 0, +1}; run in bf16 (no overflow: bf16 exponent range
                # matches fp32), then cast to the fp8 slot(s).
                sgn = wst_p.tile([P, N_shard], BF16, tag="sgn")
                nc.vector.tensor_scalar(
                    sgn[:], wst[:], 1e30, 1.0,
                    mybir.AluOpType.mult, mybir.AluOpType.min,
                )
                nc.vector.tensor_scalar(
                    sgn[:], sgn[:], -1.0, None, mybir.AluOpType.max
                )
                if k < K8C:
                    nc.vector.tensor_copy(wq[:, k, :], sgn[:])
                else:
                    s0 = K8C + 2 * (k - K8C)
                    nc.vector.tensor_copy(wq[:, s0, :], sgn[:])
                    nc.vector.tensor_copy(wq[:, s0 + 1, :], sgn[:])

            # ---------------- Phase B: alpha = mean|W| over all cores ------
            asum = small_p.tile([P, 1], F32)
            nc.vector.reduce_sum(asum[:], acc[:], axis=mybir.AxisListType.X)
            ones_pp = small_p.tile([P, P], F32)
            nc.vector.memset(ones_pp[:], 1.0)
            aps = aps_p.tile([P, 1], F32)
            # ones^T @ asum: sum over partitions, broadcast to all partitions
            nc.tensor.matmul(aps[:], ones_pp[:], asum[:], start=True, stop=True)
            part_sum = small_p.tile([P, 1], F32)
            nc.vector.tensor_copy(part_sum[:], aps[:])

            cc_in = dram_p.tile([P, 1], F32)
            cc_out = dram_p.tile(
                [P, 1], F32, addr_space="Shared" if n_cores > 4 else "Local"
            )
            nc.sync.dma_start(cc_in[:], part_sum[:])
            if skip_cc:
                nc.sync.dma_start(cc_out[:], cc_in[:])
            else:
                nc.gpsimd.collective_compute(
                    "AllReduce",
                    mybir.AluOpType.add,
                    replica_groups=[list(range(n_cores))],
                    ins=[cc_in.opt()],
                    outs=[cc_out.opt()],
                )
            gsum = small_p.tile([P, 1], F32)
            nc.sync.dma_start(gsum[:], cc_out[:])
            alpha = small_p.tile([P, 1], F32)
            nc.vector.tensor_scalar_mul(alpha[:], gsum[:], 1.0 / float(n_total_weight))

            # bias: build a [128, N_shard] bf16 broadcast of bias*alpha ONCE
            # (ones-matmul broadcast); fused into the per-block PSUM->SBUF op
            bias_sb = small_p.tile([1, N_shard], F32)
            nc.sync.dma_start(bias_sb[:], bias_d[None, :])
            ones_row = small_p.tile([1, P], F32)
            nc.vector.memset(ones_row[:], 1.0)
            bias_bc = small_p.tile([P, N_shard], BF16)
            if fuse_bias:
                nc.vector.tensor_scalar_mul(bias_sb[:], bias_sb[:], alpha[:1, :])
                for n in range(NB):
                    bps = ps_p.tile([P, N_TILE], F32, tag="ps", name=f"bps{n}")
                    nc.tensor.matmul(
                        bps[:],
                        ones_row[:],
                        bias_sb[:, ds(n * N_TILE, N_TILE)],
                        start=True,
                        stop=True,
                    )
                    nc.vector.tensor_copy(bias_bc[:, ds(n * N_TILE, N_TILE)], bps[:])

            # ---------------- Phase C: main matmul -------------------------
            total_blocks = reps * M_BLOCKS
            NSTAT = NSLOT // 2  # DR stationaries per block

            def emit_in_dma(m, tag):
                xst = xs_p.tile([P, NSLOT, P], F8, tag="xs", name=f"xs{tag}")
                nc.sync.dma_start(xst[:], xs_d[m])
                return xst

            if no_indma:
                xst_fixed = xs_p.tile([P, NSLOT, P], F8, tag="xs", name="xsfix")
                nc.vector.memset(xst_fixed[:], 0.0)
                pending = [xst_fixed] * total_blocks
            else:
                # prefetch depth 2: xs(b+2)'s transfer starts as soon as
                # xs(b-1)'s slot frees, a full block before it's needed
                pending = [
                    emit_in_dma(i % M_BLOCKS, f"b{i}")
                    for i in range(min(2, total_blocks))
                ]
            for bi in range(total_blocks):
                m = bi % M_BLOCKS
                xst = pending[bi] if no_indma else pending.pop(0)

                psums = [
                    ps_p.tile([P, N_TILE], F32, tag="ps", name=f"ps{n}")
                    for n in range(NB)
                ]
                for s in range(NSTAT):
                    lhsT = xst[:, 2 * s : 2 * s + 2, :]
                    last = s == NSTAT - 1
                    for n in range(NB):
                        nc.tensor.matmul(
                            psums[n][:],
                            lhsT,
                            wq[:, 2 * s : 2 * s + 2, ds(n * N_TILE, N_TILE)],
                            start=s == 0,
                            stop=last,
                            perf_mode=DR,
                        )

                if bi + 2 < total_blocks and not no_indma:
                    pending.append(
                        emit_in_dma((bi + 2) % M_BLOCKS, f"b{bi + 2}")
                    )

                if no_drain:
                    continue
                osb = out_p.tile([P, N_shard], F32, tag="osb")
                for n in range(NB):
                    if fuse_bias:
                        nc.vector.scalar_tensor_tensor(
                            osb[:, ds(n * N_TILE, N_TILE)],
                            psums[n][:],
                            alpha[:],
                            bias_bc[:, ds(n * N_TILE, N_TILE)],
                            mybir.AluOpType.mult,
                            mybir.AluOpType.add,
                        )
                    else:
                        nc.vector.tensor_scalar_mul(
                            osb[:, ds(n * N_TILE, N_TILE)], psums[n][:], alpha[:]
                        )
                # output store on the Activation HWDGE queue so input
                # prefetches never queue behind 1MB output bursts on SP
                nc.scalar.dma_start(out_d[m * P : (m + 1) * P, :], osb[:])

    if legalize:
        legalize_waits(nc)  # required for walrus; CoreSim chokes on raw NoOps
    elide_redundant_ldweights(nc)
    return nc


def _host_prepare(x: np.ndarray, weight: np.ndarray, bias: np.ndarray,
                  n_cores: int, k8_chunks: int):
    """Host-side dtype casts + layout (no arithmetic beyond rounding):
    x -> per-block fp8/bf16 SBUF images (shared across cores), W^T -> bf16
    per-core shards."""
    lead_shape = x.shape[:-1]
    K = x.shape[-1]
    N = weight.shape[0]
    M = int(np.prod(lead_shape))
    assert weight.shape == (N, K) and bias.shape == (N,)
    assert N % n_cores == 0
    N_shard = N // n_cores
    K8 = k8_chunks * P
    KBC = K // P - k8_chunks
    NSLOT = k8_chunks + 2 * KBC
    M_BLOCKS = M // P

    x2 = np.ascontiguousarray(x.reshape(M, K).astype(np.float32, copy=False))
    f8 = ml_dtypes.float8_e4m3
    xs = np.empty((M_BLOCKS, P, NSLOT, P), dtype=f8)
    # [mb, m, c, p] -> [mb, p, c, m]
    if k8_chunks:
        a = x2[:, :K8].astype(f8)
        xs[:, :, :k8_chunks, :] = a.reshape(M_BLOCKS, P, k8_chunks, P).transpose(
            0, 3, 2, 1
        )
    if KBC:
        hi = x2[:, K8:].astype(f8)
        lo = (x2[:, K8:] - hi.astype(np.float32)).astype(f8)
        xs[:, :, k8_chunks::2, :] = hi.reshape(M_BLOCKS, P, KBC, P).transpose(
            0, 3, 2, 1
        )
        xs[:, :, k8_chunks + 1 :: 2, :] = lo.reshape(
            M_BLOCKS, P, KBC, P
        ).transpose(0, 3, 2, 1)

    w = weight.astype(np.float32, copy=False)
    in_maps = []
    for c in range(n_cores):
        wTc = np.ascontiguousarray(
            w[c * N_shard : (c + 1) * N_shard, :].T.astype(ml_dtypes.bfloat16)
        )
        bc = np.ascontiguousarray(bias[c * N_shard : (c + 1) * N_shard]).astype(
            np.float32, copy=False
        )
        in_maps.append({"xs": xs, "wT": wTc, "bias": bc})
    return in_maps, M, K, N, N_shard, lead_shape


def run_bitnet(
    x: np.ndarray,
    weight: np.ndarray,
    bias: np.ndarray,
    n_cores: int = 8,
    k8_chunks: int = K8_CHUNKS_DEFAULT,
    trace: bool = False,
):
    """Host driver: shard, run on n_cores, gather. x: [..., K], weight: [N, K]."""
    from concourse.bass_utils import run_bass_kernel_spmd

    in_maps, M, K, N, N_shard, lead_shape = _host_prepare(
        x, weight, bias, n_cores, k8_chunks
    )
    nc = build_bitnet_nc(M, K, N_shard, N * K, n_cores=n_cores,
                         k8_chunks=k8_chunks)
    res = run_bass_kernel_spmd(
        nc, in_maps, core_ids=list(range(n_cores)), trace=trace
    )
    out = np.empty((M, N), dtype=np.float32)
    for c in range(n_cores):
        out[:, c * N_shard : (c + 1) * N_shard] = res.results[c]["out"]
    return out.reshape(*lead_shape, N), res


_RUNNER_CACHE: dict = {}


def _make_runner(nc, n_cores, in_map_names=None):
    """Compile a sharded PJRT executor for the given Bass program."""
    import jax
    import jax.numpy as jnp
    from jax.sharding import Mesh, NamedSharding, PartitionSpec
    from jax.experimental.shard_map import shard_map

    from concourse import bass2jax
    from concourse.bass2jax import _bass_exec_p, partition_id_tensor

    bass2jax.install_neuronx_cc_hook()
    partition_name = nc.partition_id_tensor.name if nc.partition_id_tensor else None
    in_names, out_names, out_avals, zero_outs = [], [], [], []
    for alloc in nc.m.functions[0].allocations:
        if not isinstance(alloc, mybir.MemoryLocationSet):
            continue
        name = alloc.memorylocations[0].name
        if alloc.kind == "ExternalInput":
            if name != partition_name:
                in_names.append(name)
        elif alloc.kind == "ExternalOutput":
            shape = tuple(alloc.tensor_shape)
            dtype = mybir.dt.np(alloc.dtype)
            out_names.append(name)
            out_avals.append(jax.core.ShapedArray(shape, dtype))
            zero_outs.append(np.zeros(shape, dtype))
    n_params = len(in_names)
    n_outs = len(out_avals)
    param_names = list(in_names)
    in_names = in_names + out_names
    if partition_name is not None:
        in_names.append(partition_name)
    donate = tuple(range(n_params, n_params + n_outs))

    def _body(*args):
        operands = list(args)
        if partition_name is not None:
            operands.append(partition_id_tensor())
        return tuple(
            _bass_exec_p.bind(
                *operands,
                out_avals=tuple(out_avals),
                in_names=tuple(in_names),
                out_names=tuple(out_names),
                lowering_input_output_aliases=(),
                sim_require_finite=True,
                sim_require_nnan=True,
                nc=nc,
            )
        )

    devices = jax.devices()[:n_cores]
    mesh = Mesh(np.asarray(devices), ("core",))
    sh = NamedSharding(mesh, PartitionSpec("core"))
    sharded = jax.jit(
        shard_map(
            _body,
            mesh=mesh,
            in_specs=(PartitionSpec("core"),) * (n_params + n_outs),
            out_specs=(PartitionSpec("core"),) * len(out_names),
            check_rep=False,
        ),
        donate_argnums=donate,
        keep_unused=True,
    )
    zfns = [
        jax.jit(
            lambda shp=(n_cores * z.shape[0], *z.shape[1:]),
            dt=z.dtype: jnp.zeros(shp, dt),
            out_shardings=sh,
        )
        for z in zero_outs
    ]
    return sharded, param_names, out_names, out_avals, sh, zfns


def _cached_pjrt_run(M, K, N_shard, n_cores, k8_chunks, in_maps):
    """Compile-once-per-shape PJRT executor; repeat kernel() calls skip the
    multi-minute NEFF rebuild and only pay transfer + execution."""
    import jax

    key = (M, K, N_shard, n_cores, k8_chunks)
    if key not in _RUNNER_CACHE:
        nc = build_bitnet_nc(M, K, N_shard, N_shard * n_cores * K,
                             n_cores=n_cores, k8_chunks=k8_chunks)
        _RUNNER_CACHE[key] = _make_runner(nc, n_cores)

    sharded, param_names, out_names, out_avals, sh, zfns = _RUNNER_CACHE[key]

    concat_in = [
        jax.device_put(
            np.concatenate(
                [np.asarray(in_maps[c][nm]) for c in range(n_cores)], 0
            ),
            sh,
        )
        for nm in param_names
    ]
    out_arrs = sharded(*concat_in, *[f() for f in zfns])
    oi = out_names.index("out")
    glob = np.asarray(out_arrs[oi]).reshape(n_cores, *out_avals[oi].shape)
    return [glob[c] for c in range(n_cores)]


def kernel(x: np.ndarray, weight: np.ndarray, bias: np.ndarray) -> np.ndarray:
    n_cores = 8
    k8_chunks = K8_CHUNKS_DEFAULT
    in_maps, M, K, N, N_shard, lead_shape = _host_prepare(
        x, weight, bias, n_cores, k8_chunks
    )
    shards = _cached_pjrt_run(M, K, N_shard, n_cores, k8_chunks, in_maps)
    out = np.empty((M, N), dtype=np.float32)
    for c in range(n_cores):
        out[:, c * N_shard : (c + 1) * N_shard] = shards[c]
    return out.reshape(*lead_shape, N)


def run_bitnet_timed(
    x: np.ndarray,
    weight: np.ndarray,
    bias: np.ndarray,
    n_cores: int = 8,
    nsplits: int = 2,  # kept for test.py signature compat; unused
    reps: int = 4,
    rounds: int = 6,
    k8_chunks: int = K8_CHUNKS_DEFAULT,
):
    """Like run_bitnet, but measures HW time via the reps-difference method:
    build the kernel once plain and once with the main loop unrolled `reps`
    times, time single dispatches of each (min over `rounds`), and divide the
    delta by reps-1.  This cancels the multi-ms, noisy axon dispatch floor.
    Returns (out, per_exec_seconds, diag)."""
    import time

    import jax

    in_maps, M, K, N, N_shard, lead_shape = _host_prepare(
        x, weight, bias, n_cores, k8_chunks
    )

    def runner_for(reps_):
        nc = build_bitnet_nc(M, K, N_shard, N * K, n_cores=n_cores,
                             k8_chunks=k8_chunks, reps=reps_)
        sharded, param_names, out_names, out_avals, sh, zfns = _make_runner(
            nc, n_cores
        )
        concat_in = [
            jax.device_put(
                np.concatenate(
                    [np.asarray(in_maps[c][nm]) for c in range(n_cores)], 0
                ),
                sh,
            )
            for nm in param_names
        ]

        def run_once():
            z = [f() for f in zfns]
            jax.block_until_ready(z)
            t0 = time.perf_counter()
            o = sharded(*concat_in, *z)
            jax.block_until_ready(o)
            return time.perf_counter() - t0, o

        return run_once, out_names, out_avals

    run1, out_names, out_avals = runner_for(1)
    t_warm, out_arrs = run1()  # includes NEFF compile+load

    runR, _, _ = runner_for(reps)
    runR()  # warmup/compile

    t1s, tRs = [], []
    for _ in range(rounds):
        t1s.append(run1()[0])
        tRs.append(runR()[0])
    t1 = min(t1s)
    tR = min(tRs)
    per_exec = (tR - t1) / (reps - 1)
    diag = {"t1_min": t1, "tR_min": tR, "t1s": t1s, "tRs": tRs}

    oi = out_names.index("out")
    glob = np.asarray(out_arrs[oi]).reshape(n_cores, M, N_shard)
    out = np.empty((M, N), dtype=np.float32)
    for c in range(n_cores):
        out[:, c * N_shard : (c + 1) * N_shard] = glob[c]
    return out.reshape(*lead_shape, N), per_exec, diag
